# revision 6
# baseline (speedup 1.0000x reference)
"""Trainium2 Bass kernel for nn_KromHCHeadMixer.

Math (per batch element b):
    ctx   = mean_h x[b]                                  (64,)
    p_i   = sigmoid(relu(ctx @ W1[i] + b1[i]) @ w2d_i + b2d_i)   i = 0..2
            where w2d_i = W2[i,:,0] - W2[i,:,1]  (softmax over 2 == sigmoid of diff)
    U_i   = [[p_i, q_i], [q_i, p_i]],  q_i = 1 - p_i
    H[b]  = U_0 (x) U_1 (x) U_2        (kron),  out[b] = H[b] @ x[b]

Kernel strategy (pure data parallel over 8 cores, batch-sharded):
  - x converted to fp16 on host (halves DMA, enables DVE 2x modes).
  - batch tile of 128 rows on SBUF partitions, 512 features on free dim.
  - out computed as a 3-level butterfly with per-partition scalars
    (scalar_tensor_tensor), never materializing the 8x8 matmul.
  - H computed as V @ PAT on the tensor engine, where V[b, m] are the 8
    subset products of p_i/q_i and PAT is a constant 0/1 (8, 64) matrix.
"""

import numpy as np

BS, NH, HD, K = 131072, 8, 64, 3
FREE = NH * HD  # 512
NCORES = 8
PER = BS // NCORES  # 16384 rows per core
TILE_P = 128


def _build_nc(per_rows: int):
    import concourse.bass as bass
    import concourse.mybir as mybir
    from concourse import bacc, tile

    dt = mybir.dt
    f16, f32 = dt.float16, dt.float32
    AF = mybir.ActivationFunctionType
    OP = mybir.AluOpType

    ntiles = per_rows // TILE_P

    nc = bacc.Bacc(None, target_bir_lowering=False, debug=False)
    x_in = nc.dram_tensor("x", [per_rows, FREE], f16, kind="ExternalInput")
    w1_in = nc.dram_tensor("w1", [HD, 3 * 32], f16, kind="ExternalInput")
    b1_in = nc.dram_tensor("b1", [3 * 32, 1], f32, kind="ExternalInput")
    w2_in = nc.dram_tensor("w2", [3 * 32, 6], f16, kind="ExternalInput")
    b2_in = nc.dram_tensor("b2", [6, 1], f32, kind="ExternalInput")
    pat_in = nc.dram_tensor("pat", [8, 64], f16, kind="ExternalInput")
    id_in = nc.dram_tensor("ident", [128, 128], f16, kind="ExternalInput")
    out_o = nc.dram_tensor("out", [per_rows, FREE], f16, kind="ExternalOutput")
    h_o = nc.dram_tensor("hmat", [per_rows, 64], f16, kind="ExternalOutput")

    with tile.TileContext(nc) as tc:
        with (
            tc.tile_pool(name="const", bufs=1) as cp,
            tc.tile_pool(name="xin", bufs=4) as xp,
            tc.tile_pool(name="stage", bufs=3) as sp,
            tc.tile_pool(name="outp", bufs=4) as op_,
            tc.tile_pool(name="tmp", bufs=3) as tp,
            tc.tile_pool(name="small", bufs=3) as mp,
            tc.tile_pool(name="ps_a", bufs=1, space="PSUM") as pa,
            tc.tile_pool(name="ps_h", bufs=2, space="PSUM") as ph,
        ):
            w1_t = cp.tile([HD, 96], f16, tag="w1")
            nc.sync.dma_start(w1_t[:], w1_in[:])
            b1_t = cp.tile([96, 1], f32, tag="b1")
            nc.sync.dma_start(b1_t[:], b1_in[:])
            w2_t = cp.tile([96, 6], f16, tag="w2")
            nc.sync.dma_start(w2_t[:], w2_in[:])
            b2_t = cp.tile([6, 1], f32, tag="b2")
            nc.sync.dma_start(b2_t[:], b2_in[:])
            pat_t = cp.tile([8, 64], f16, tag="pat")
            nc.sync.dma_start(pat_t[:], pat_in[:])
            id_t = cp.tile([128, 128], f16, tag="ident")
            nc.sync.dma_start(id_t[:], id_in[:])

            for i in range(ntiles):
                rows = slice(i * TILE_P, (i + 1) * TILE_P)
                xb = xp.tile([TILE_P, FREE], f16, tag="xb")
                nc.sync.dma_start(xb[:], x_in[rows, :])

                # ---- context sum over heads (mean folded into W1 on host)
                s1 = tp.tile([TILE_P, 256], f16, tag="s1")
                nc.vector.tensor_add(s1[:], xb[:, 0:256], xb[:, 256:512])
                s2 = mp.tile([TILE_P, 128], f16, tag="s2")
                nc.vector.tensor_add(s2[:], s1[:, 0:128], s1[:, 128:256])
                ctx = mp.tile([TILE_P, 64], f16, tag="ctx")
                nc.vector.tensor_add(ctx[:], s2[:, 0:64], s2[:, 64:128])

                # ---- ctx^T (64, 128) for the MLP matmuls
                ctxT_ps = pa.tile([64, 128], f16, tag="ctxT_ps")
                nc.tensor.transpose(ctxT_ps[:], ctx[:], id_t[:])
                ctxT = mp.tile([64, 128], f16, tag="ctxT")
                nc.scalar.copy(ctxT[:], ctxT_ps[:])

                # ---- hidden = relu(W1s^T @ ctxT + b1): (96, 128)
                h_ps = pa.tile([96, 128], f32, tag="h_ps")
                nc.tensor.matmul(h_ps[:], w1_t[:], ctxT[:], start=True, stop=True)
                hh = mp.tile([96, 128], f16, tag="hh")
                nc.scalar.activation(hh[:], h_ps[:], AF.Relu, bias=b1_t[:])

                # ---- probs rows [p0,q0,p1,q1,p2,q2] = sigmoid(w2bd^T @ h + b2d)
                l_ps = pa.tile([6, 128], f32, tag="l_ps")
                nc.tensor.matmul(l_ps[:], w2_t[:], hh[:], start=True, stop=True)
                probs = mp.tile([6, 128], f16, tag="probs")
                nc.scalar.activation(probs[:], l_ps[:], AF.Sigmoid, bias=b2_t[:])

                # ---- transpose probs -> (128, 6) per-partition scalars
                pT_ps = pa.tile([TILE_P, 6], f16, tag="pT_ps")
                nc.tensor.transpose(pT_ps[:], probs[:], id_t[0:6, 0:6])
                pT = mp.tile([TILE_P, 6], f32, tag="pT")
                nc.scalar.copy(pT[:], pT_ps[:])
                p0, q0 = pT[:, 0:1], pT[:, 1:2]
                p1, q1 = pT[:, 2:3], pT[:, 3:4]
                p2, q2 = pT[:, 4:5], pT[:, 5:6]

                # ---- V[b, 4*i0+2*i1+i2] = f0[i0]*f1[i1]*f2[i2], f_i = [p_i, q_i]
                vb = mp.tile([TILE_P, 4], f16, tag="vb")
                nc.gpsimd.tensor_scalar_mul(vb[:, 0:2], pT[:, 4:6], p1)
                nc.gpsimd.tensor_scalar_mul(vb[:, 2:4], pT[:, 4:6], q1)
                vv = mp.tile([TILE_P, 8], f16, tag="vv")
                nc.gpsimd.tensor_scalar_mul(vv[:, 0:4], vb[:], p0)
                nc.gpsimd.tensor_scalar_mul(vv[:, 4:8], vb[:], q0)

                # ---- H = (V^T)^T @ PAT on PE; DMA straight out of PSUM
                vT_ps = pa.tile([8, 128], f16, tag="vT_ps")
                nc.tensor.transpose(vT_ps[:], vv[:], id_t[:])
                vT = mp.tile([8, 128], f16, tag="vT")
                nc.scalar.copy(vT[:], vT_ps[:])
                hm_ps = ph.tile([TILE_P, 64], f32, tag="hm_ps")
                nc.tensor.matmul(hm_ps[:], vT[:], pat_t[:], start=True, stop=True)
                hm = mp.tile([TILE_P, 64], f16, tag="hm")
                nc.scalar.copy(hm[:], hm_ps[:])
                nc.sync.dma_start(h_o[rows, :], hm[:])

                # ---- butterfly level 0 (head bit 2): halves of 256
                t0 = tp.tile([TILE_P, 256], f16, tag="t0")
                nc.vector.tensor_sub(t0[:], xb[:, 0:256], xb[:, 256:512])
                yy = sp.tile([TILE_P, FREE], f16, tag="yy")
                nc.vector.scalar_tensor_tensor(
                    yy[:, 0:256], t0[:], p0, xb[:, 256:512], OP.mult, OP.add
                )
                nc.vector.scalar_tensor_tensor(
                    yy[:, 256:512], t0[:], q0, xb[:, 256:512], OP.mult, OP.add
                )

                # ---- level 1 (head bit 1): stride-128 pairs
                y4 = yy[:].rearrange("p (a w) -> p a w", w=128)
                t1 = tp.tile([TILE_P, 256], f16, tag="t1")
                t1v = t1[:].rearrange("p (a w) -> p a w", w=128)
                nc.vector.tensor_sub(t1v, y4[:, 0::2, :], y4[:, 1::2, :])
                zz = sp.tile([TILE_P, FREE], f16, tag="zz")
                z4 = zz[:].rearrange("p (a w) -> p a w", w=128)
                nc.vector.scalar_tensor_tensor(
                    z4[:, 0::2, :], t1v, p1, y4[:, 1::2, :], OP.mult, OP.add
                )
                nc.vector.scalar_tensor_tensor(
                    z4[:, 1::2, :], t1v, q1, y4[:, 1::2, :], OP.mult, OP.add
                )

                # ---- level 2 (head bit 0): stride-64 pairs
                z8 = zz[:].rearrange("p (a w) -> p a w", w=64)
                t2 = tp.tile([TILE_P, 256], f16, tag="t2")
                t2v = t2[:].rearrange("p (a w) -> p a w", w=64)
                nc.vector.tensor_sub(t2v, z8[:, 0::2, :], z8[:, 1::2, :])
                ob = op_.tile([TILE_P, FREE], f16, tag="ob")
                o8 = ob[:].rearrange("p (a w) -> p a w", w=64)
                nc.vector.scalar_tensor_tensor(
                    o8[:, 0::2, :], t2v, p2, z8[:, 1::2, :], OP.mult, OP.add
                )
                nc.vector.scalar_tensor_tensor(
                    o8[:, 1::2, :], t2v, q2, z8[:, 1::2, :], OP.mult, OP.add
                )
                nc.sync.dma_start(out_o[rows, :], ob[:])

    nc.finalize()
    return nc


def _prep_consts(W1, b1, W2, b2):
    """Host-side constant preprocessing. All fp16 except biases."""
    W1 = np.asarray(W1, dtype=np.float32)
    b1 = np.asarray(b1, dtype=np.float32)
    W2 = np.asarray(W2, dtype=np.float32)
    b2 = np.asarray(b2, dtype=np.float32)

    # (3,64,32) -> (64,96), mean-over-heads 1/8 folded in
    w1s = np.concatenate([W1[i] / 8.0 for i in range(K)], axis=1)
    b1s = b1.reshape(96, 1)

    w2bd = np.zeros((96, 6), dtype=np.float32)
    b2d = np.zeros((6, 1), dtype=np.float32)
    for i in range(K):
        d = W2[i, :, 0] - W2[i, :, 1]  # (32,)
        w2bd[32 * i : 32 * (i + 1), 2 * i] = d
        w2bd[32 * i : 32 * (i + 1), 2 * i + 1] = -d
        db = b2[i, 0] - b2[i, 1]
        b2d[2 * i, 0] = db
        b2d[2 * i + 1, 0] = -db

    pat = np.zeros((8, 64), dtype=np.float32)
    for j in range(8):
        for k in range(8):
            m = (
                4 * (((j >> 2) ^ (k >> 2)) & 1)
                + 2 * (((j >> 1) ^ (k >> 1)) & 1)
                + ((j ^ k) & 1)
            )
            pat[m, 8 * j + k] = 1.0

    ident = np.eye(128, dtype=np.float32)

    return {
        "w1": w1s.astype(np.float16),
        "b1": b1s,
        "w2": w2bd.astype(np.float16),
        "b2": b2d,
        "pat": pat.astype(np.float16),
        "ident": ident.astype(np.float16),
    }


def run_on_hw(inputs, trace=False, trace_kwargs=None):
    """Shard, run on 8 NeuronCores, gather. Returns (out, H, exec_time_ns)."""
    from concourse.bass_utils import run_bass_kernel_spmd

    x = np.asarray(inputs["x"], dtype=np.float32).reshape(BS, FREE)
    consts = _prep_consts(inputs["W1"], inputs["b1"], inputs["W2"], inputs["b2"])

    nc = _build_nc(PER)
    in_maps = []
    for c in range(NCORES):
        shard = x[c * PER : (c + 1) * PER].astype(np.float16)
        in_maps.append({"x": shard, **consts})

    kw = {}
    if trace:
        kw["trace"] = True
        if trace_kwargs:
            kw.update(trace_kwargs)
    res = run_bass_kernel_spmd(nc, in_maps, list(range(NCORES)), **kw)

    out = np.concatenate(
        [res.results[c]["out"].astype(np.float32) for c in range(NCORES)], axis=0
    ).reshape(BS, NH, HD)
    H = np.concatenate(
        [res.results[c]["hmat"].astype(np.float32) for c in range(NCORES)], axis=0
    ).reshape(BS, NH, NH)
    return out, H, res.exec_time_ns


def kernel(x, W1, b1, W2, b2):
    out, H, _ = run_on_hw({"x": x, "W1": W1, "b1": b1, "W2": W2, "b2": b2})
    return out, H
